# revision 6
# baseline (speedup 1.0000x reference)
"""Trainium2 Bass kernel for nn_DINOBevAligner (BEVFormer-style view aligner).

Strategy (8 NeuronCores, channel-sharded):
  - Channels C=768 are sharded 8 x 96. Every core holds ALL views/tokens for
    its channel slice, so the bilinear gather needs no cross-core traffic.
  - Pre-LN stats (sum/sumsq over C per token) are computed per-slice and
    combined with one small AllReduce (128x132 f32).
  - The bilinear gather + pillar mask + view weighting is expressed as a
    small set of dense TensorEngine matmuls: tokens are stored x-major
    (n' = x*37 + y) in 128-token tiles; queries are globally ordered by BEV
    azimuth so each (view, token-tile) touches a contiguous run of query
    columns.  Host builds the sparse->dense weight blocks (bf16).
  - Post-LN over C needs Sum_c num^2: per-slice ones-matmul partials are
    combined with a second small AllReduce (2560 f32).  Mean over C of the
    fused feature is exactly 0 (LayerNorm output sums to zero), so no mean
    correction is needed.
  - The grouped softmax reducer (C=768 -> 256, groups of 3) is one more
    tiny matmul per query tile; gamma/softmax(logits) fold into its weights.
Host work is limited to projection/index/weight-matrix construction (the
sampling-operator descriptors, ~100KB derived from the 6 4x4 matrices) and
input/output relayout; all tensor math runs on device.
"""
import sys

sys.path.insert(0, "/opt/trn_rl_repo")

import numpy as np
import ml_dtypes

BEV_H, BEV_W = 50, 50
D_PILLAR = 4
PC = (-51.2, -51.2, -5.0, 51.2, 51.2, 3.0)
S_IMG = 518.0
LN_EPS = 1e-5
FUSE_EPS = 1e-6
C_CTX = 256
Q = BEV_H * BEV_W
QP = 2560
NQT = QP // 128
TOK_TILE = 128
MAX_N = 512
V = 6
C = 768
CS = C // 8          # 96 channels per core
KS = C_CTX // 8      # 32 output channels per core
NCORE = 8


# ----------------------------------------------------------------- host math
def _projection_np(lidar2img):
    dt = np.float32
    Z = int(round(PC[5] - PC[2]))
    zs = (np.linspace(0.5, Z - 0.5, D_PILLAR, dtype=dt) / dt(Z))[:, None, None]
    xs = (np.linspace(0.5, BEV_W - 0.5, BEV_W, dtype=dt) / dt(BEV_W))[None, None, :]
    ys = (np.linspace(0.5, BEV_H - 0.5, BEV_H, dtype=dt) / dt(BEV_H))[None, :, None]
    x, y, z = np.broadcast_arrays(xs, ys, zs)
    ref = np.stack([x, y, z], axis=-1).reshape(D_PILLAR, Q, 3).astype(dt)
    ref = ref * np.array([PC[3] - PC[0], PC[4] - PC[1], PC[5] - PC[2]], dt) \
        + np.array([PC[0], PC[1], PC[2]], dt)
    ref4 = np.concatenate([ref, np.ones_like(ref[..., :1])], axis=-1)
    pts = np.einsum('bvij,dqj->bdvqi', lidar2img.astype(dt), ref4)
    zc = pts[..., 2]
    valid = zc > 1e-5
    uv = pts[..., :2] / np.maximum(zc, dt(1e-5))[..., None] / dt(S_IMG)
    u, v = uv[..., 0], uv[..., 1]
    valid = valid & (u > 0.0) & (u < 1.0) & (v > 0.0) & (v < 1.0)
    tr = lambda a: np.transpose(a, (0, 2, 3, 1))
    return tr(u), tr(v), tr(valid)


def build_plan(lidar2img, patch_h, patch_w):
    dt = np.float32
    Hp, Wp = int(patch_h), int(patch_w)
    u, v, valid = _projection_np(lidar2img)
    u, v, valid = u[0], v[0], valid[0]              # (V,Q,D)

    x_p = (u * dt(S_IMG) + dt(0.5)) / dt(S_IMG) * dt(Wp) - dt(0.5)
    y_p = (v * dt(S_IMG) + dt(0.5)) / dt(S_IMG) * dt(Hp) - dt(0.5)
    x0 = np.floor(x_p); fx = x_p - x0; x0 = x0.astype(np.int64)
    y0 = np.floor(y_p); fy = y_p - y0; y0 = y0.astype(np.int64)
    m = valid.astype(dt)
    cnt = m.sum(axis=-1)

    toks = np.full((V, Q, D_PILLAR, 4), -1, dtype=np.int64)
    wts = np.zeros((V, Q, D_PILLAR, 4), dtype=dt)
    ci = 0
    for dx in (0, 1):
        for dy in (0, 1):
            xi, yi = x0 + dx, y0 + dy
            inb = (xi >= 0) & (xi < Wp) & (yi >= 0) & (yi < Hp)
            w = np.where(dx, fx, 1 - fx) * np.where(dy, fy, 1 - fy) * inb.astype(dt)
            w = w * m
            n_xmaj = np.clip(xi, 0, Wp - 1) * Hp + np.clip(yi, 0, Hp - 1)
            live = (w != 0) & inb
            toks[..., ci] = np.where(live, n_xmaj, -1)
            wts[..., ci] = np.where(live, w, 0)
            ci += 1

    qy, qx = np.divmod(np.arange(Q), BEV_W)
    az = np.arctan2(qy - (BEV_H - 1) / 2.0, qx - (BEV_W - 1) / 2.0)
    perm = np.argsort(az, kind='stable').astype(np.int64)
    pos_of = np.empty(Q, dtype=np.int64)
    pos_of[perm] = np.arange(Q)

    NT_V = (Hp * Wp + TOK_TILE - 1) // TOK_TILE

    cp = np.zeros((QP, V), dtype=dt)
    cp[:Q] = cnt.T[perm]
    cnt_perm = cp.reshape(NQT, 128, V).transpose(1, 0, 2).copy()

    tk = toks.reshape(V, Q, 16)
    wt = wts.reshape(V, Q, 16)
    mms, wblocks, woff = [], [], 0
    for vv in range(V):
        live_q = np.where((wt[vv] != 0).any(axis=1))[0]
        if live_q.size == 0:
            continue
        pos = pos_of[live_q]
        order = np.argsort(pos)
        live_q, pos = live_q[order], pos[order]
        tiles_of = tk[vv, live_q] // TOK_TILE
        for t in range(NT_V):
            touch = (tiles_of == t).any(axis=1)
            idx = np.where(touch)[0]
            if idx.size == 0:
                continue
            runs = []
            start = prev = idx[0]
            for j in idx[1:]:
                pj = pos[j]
                if (pos[prev] + 1 != pj or pj % MAX_N == 0
                        or pj - pos[start] >= MAX_N
                        or pj // MAX_N != pos[start] // MAX_N):
                    runs.append((start, prev))
                    start = j
                prev = j
            runs.append((start, prev))
            merged = []
            for (a, b) in runs:
                if merged:
                    pa, pb = merged[-1]
                    if (pos[a] // MAX_N == pos[pa] // MAX_N
                            and pos[b] - pos[pa] < MAX_N
                            and pos[a] - pos[pb] <= 8):
                        merged[-1] = (pa, b)
                        continue
                merged.append((a, b))
            for (a, b) in merged:
                p0, p1 = int(pos[a]), int(pos[b])
                ncols = p1 - p0 + 1
                W_blk = np.zeros((TOK_TILE, ncols), dtype=dt)
                sel = np.where((pos >= p0) & (pos <= p1))[0]
                for j in sel:
                    col = pos[j] - p0
                    for c16 in range(16):
                        n = tk[vv, live_q[j], c16]
                        if n >= 0 and n // TOK_TILE == t:
                            W_blk[n % TOK_TILE, col] += wt[vv, live_q[j], c16]
                mms.append((vv, t, p0, ncols, woff))
                wblocks.append(W_blk)
                woff += ncols

    wmat = (np.concatenate(wblocks, axis=1) if wblocks
            else np.zeros((TOK_TILE, 1), dtype=dt))
    return dict(perm=perm, cnt_perm=cnt_perm, wmat=wmat, mms=mms, NT_V=NT_V)


def retile_tokens(last_tokens, NT_V, Hp, Wp):
    B, Vv, N, Cc = last_tokens.shape
    out = np.zeros((128, Vv * NT_V, Cc), dtype=np.float32)
    for vv in range(Vv):
        t = last_tokens[0, vv].reshape(Hp, Wp, Cc).transpose(1, 0, 2).reshape(N, Cc)
        pad = np.zeros((NT_V * 128, Cc), dtype=np.float32)
        pad[:N] = t
        out[:, vv * NT_V:(vv + 1) * NT_V, :] = \
            pad.reshape(NT_V, 128, Cc).transpose(1, 0, 2)
    return out


# -------------------------------------------------------------- bass program
def build_program(NTT, WCOLS, mms):
    import concourse.bass as bass
    import concourse.bacc as bacc
    import concourse.tile as tile
    from concourse import mybir

    f32 = mybir.dt.float32
    bf16 = mybir.dt.bfloat16
    AF = mybir.ActivationFunctionType
    ALU = mybir.AluOpType

    nc = bacc.Bacc("TRN2", target_bir_lowering=False, debug=False,
                   num_devices=NCORE)

    tok_d = nc.dram_tensor("tok", [128, NTT * CS], f32, kind="ExternalInput")
    wmat_d = nc.dram_tensor("wmat", [128, WCOLS], bf16, kind="ExternalInput")
    cnt_d = nc.dram_tensor("cnt", [128, NQT * V], f32, kind="ExternalInput")
    rowc_d = nc.dram_tensor("rowc", [1, 640], f32, kind="ExternalInput")
    m1m_d = nc.dram_tensor("m1mask", [CS, KS], f32, kind="ExternalInput")
    onesw_d = nc.dram_tensor("onesw", [CS, KS], bf16, kind="ExternalInput")
    zrow_d = nc.dram_tensor("zrow", [1, 512], bf16, kind="ExternalInput")
    out_d = nc.dram_tensor("out", [128, NQT * KS], f32, kind="ExternalOutput")

    # last matmul touching each 512-col psum bank -> stop flag
    last_in_bank = {}
    for i, (vv, t, p0, ncols, woff) in enumerate(mms):
        last_in_bank[p0 // MAX_N] = i
    stop_idx = set(last_in_bank.values())
    banks_touched = set(last_in_bank.keys())

    NBN = 5                       # token tiles per bn_stats chunk (free 480)
    nchunks = (NTT + NBN - 1) // NBN

    with tile.TileContext(nc) as tc:
        with (
            tc.tile_pool(name="big", bufs=1) as big,
            tc.tile_pool(name="small", bufs=1) as small,
            tc.tile_pool(name="psum", bufs=1, space="PSUM") as psum,
            tc.tile_pool(name="dram", bufs=1, space="DRAM") as dram,
        ):
            # ---------------- input DMAs
            tokS = big.tile([128, NTT, CS], f32, tag="tokS")
            tok_v = tok_d.ap().rearrange("p (t c) -> p t c", c=CS)
            for ch in range(nchunks):
                t0, t1 = ch * NBN, min((ch + 1) * NBN, NTT)
                nc.sync.dma_start(out=tokS[:, t0:t1, :], in_=tok_v[:, t0:t1, :])
            wS = big.tile([128, WCOLS], bf16, tag="wS")
            nc.sync.dma_start(out=wS[:], in_=wmat_d.ap())
            cntS = small.tile([128, NQT, V], f32, tag="cntS")
            nc.sync.dma_start(out=cntS[:],
                              in_=cnt_d.ap().rearrange("p (t v) -> p t v", v=V))
            rowS = small.tile([1, 640], f32, tag="rowS")
            nc.sync.dma_start(out=rowS[:], in_=rowc_d.ap())
            m1S = small.tile([CS, KS], f32, tag="m1S")
            nc.sync.dma_start(out=m1S[:], in_=m1m_d.ap())
            oneswS = small.tile([CS, KS], bf16, tag="oneswS")
            nc.sync.dma_start(out=oneswS[:], in_=onesw_d.ap())
            zrowS = small.tile([1, 512], bf16, tag="zrowS")
            nc.sync.dma_start(out=zrowS[:], in_=zrow_d.ap())
            # broadcast w_view (rowc[288:294]) across partitions
            wvb = small.tile([128, V], f32, tag="wvb")
            wv_bcast = bass.AP(tensor=rowc_d, offset=288,
                               ap=[[0, 128], [1, V]])
            nc.gpsimd.dma_start(out=wvb[:], in_=wv_bcast)
            # softplus(x) = ln(1 + exp(x))  (no HW softplus table)
            nc.scalar.activation(out=wvb[:], in_=wvb[:], func=AF.Exp)
            nc.vector.tensor_scalar_add(wvb[:], wvb[:], 1.0)
            nc.scalar.activation(out=wvb[:], in_=wvb[:], func=AF.Ln)

            epsT = small.tile([128, 1], f32, tag="epsT")
            nc.vector.memset(epsT[:], LN_EPS)

            # ---------------- pre-LN stats (per 96-channel slice)
            # bn_stats collapses its whole free dim -> one call per token tile
            bn6 = small.tile([128, NTT, 6], f32, tag="bn6")
            for t in range(NTT):
                nc.vector.bn_stats(out=bn6[:, t, :], in_=tokS[:, t, :])
            part = small.tile([128, 2, NTT], f32, tag="part")
            t1m = small.tile([128, NTT], f32, tag="t1m")
            t2m = small.tile([128, NTT], f32, tag="t2m")
            # S = ce*me + co*mo
            nc.vector.tensor_tensor(out=t1m[:], in0=bn6[:, :, 0], in1=bn6[:, :, 1],
                                    op=ALU.mult)
            nc.vector.tensor_tensor(out=t2m[:], in0=bn6[:, :, 3], in1=bn6[:, :, 4],
                                    op=ALU.mult)
            nc.vector.tensor_tensor(out=part[:, 0, :], in0=t1m[:], in1=t2m[:],
                                    op=ALU.add)
            # SS = M2e + M2o + (ce*me)*me + (co*mo)*mo
            nc.vector.tensor_tensor(out=t1m[:], in0=t1m[:], in1=bn6[:, :, 1],
                                    op=ALU.mult)
            nc.vector.tensor_tensor(out=t2m[:], in0=t2m[:], in1=bn6[:, :, 4],
                                    op=ALU.mult)
            nc.vector.tensor_tensor(out=t1m[:], in0=t1m[:], in1=bn6[:, :, 2],
                                    op=ALU.add)
            nc.vector.tensor_tensor(out=t2m[:], in0=t2m[:], in1=bn6[:, :, 5],
                                    op=ALU.add)
            nc.vector.tensor_tensor(out=part[:, 1, :], in0=t1m[:], in1=t2m[:],
                                    op=ALU.add)

            # ---------------- all-reduce #1 (LN stats over the 8 slices)
            st_in = dram.tile([128, 2 * NTT], f32, tag="st_in")
            st_out = dram.tile([128, 2 * NTT], f32, tag="st_out")
            nc.sync.dma_start(out=st_in[:], in_=part[:].rearrange("p a t -> p (a t)"))
            nc.gpsimd.collective_compute(
                "AllReduce", ALU.add,
                replica_groups=[list(range(NCORE))],
                ins=[st_in[:].opt()], outs=[st_out[:].opt()],
            )
            red = small.tile([128, 2, NTT], f32, tag="red")
            nc.sync.dma_start(out=red[:],
                              in_=st_out[:].rearrange("p (a t) -> p a t", a=2))

            # mu, inv, scale, nbias
            mu = small.tile([128, NTT], f32, tag="mu")
            varT = small.tile([128, NTT], f32, tag="varT")
            nc.vector.tensor_scalar_mul(mu[:], red[:, 0, :], 1.0 / C)
            nc.vector.tensor_scalar_mul(varT[:], red[:, 1, :], 1.0 / C)
            nc.vector.tensor_tensor(out=t1m[:], in0=mu[:], in1=mu[:], op=ALU.mult)
            nc.vector.tensor_tensor(out=varT[:], in0=varT[:], in1=t1m[:],
                                    op=ALU.subtract)
            nc.scalar.activation(out=varT[:], in_=varT[:], func=AF.Sqrt,
                                 bias=epsT[:], scale=1.0)
            nc.vector.reciprocal(out=varT[:], in_=varT[:])   # varT = 1/sqrt(var+eps)
            scale = small.tile([128, NTT], f32, tag="scale")
            NTV = NTT // V
            nc.vector.tensor_tensor(
                out=scale[:].rearrange("p (v t) -> p v t", v=V),
                in0=varT[:].rearrange("p (v t) -> p v t", v=V),
                in1=wvb[:].unsqueeze(2).broadcast_to([128, V, NTV]),
                op=ALU.mult)
            nbias = small.tile([128, NTT], f32, tag="nbias")
            nc.vector.tensor_tensor(out=nbias[:], in0=mu[:], in1=scale[:],
                                    op=ALU.mult)
            nc.vector.tensor_scalar_mul(nbias[:], nbias[:], -1.0)

            # ---------------- normalize + cast to bf16 (chunked per view)
            xb = big.tile([128, NTT, CS], bf16, tag="xb")
            for vv in range(V):
                t0, t1 = vv * NTV, (vv + 1) * NTV
                nc.vector.tensor_tensor(
                    out=tokS[:, t0:t1, :], in0=tokS[:, t0:t1, :],
                    in1=scale[:, t0:t1].unsqueeze(2).broadcast_to([128, NTV, CS]),
                    op=ALU.mult)
                nc.vector.tensor_tensor(
                    out=xb[:, t0:t1, :], in0=tokS[:, t0:t1, :],
                    in1=nbias[:, t0:t1].unsqueeze(2).broadcast_to([128, NTV, CS]),
                    op=ALU.add)

            # ---------------- den and r = 1/(den+eps)
            prod = small.tile([128, NQT, V], f32, tag="prod")
            nc.vector.tensor_tensor(
                out=prod[:], in0=cntS[:],
                in1=wvb[:].unsqueeze(1).broadcast_to([128, NQT, V]),
                op=ALU.mult)
            den = small.tile([128, NQT], f32, tag="den")
            nc.vector.tensor_reduce(out=den[:], in_=prod[:],
                                    axis=mybir.AxisListType.X, op=ALU.add)
            rq = small.tile([128, NQT], f32, tag="rq")
            nc.vector.tensor_scalar_add(rq[:], den[:], FUSE_EPS)
            nc.vector.reciprocal(out=rq[:], in_=rq[:])

            # ---------------- gather matmuls into PSUM accumulator
            accP = psum.tile([128, QP], f32, tag="bigp")
            for j in range(QP // MAX_N):
                nc.tensor.matmul(accP[0:CS, j * MAX_N:(j + 1) * MAX_N],
                                 lhsT=zrowS[0:1, 0:CS], rhs=zrowS[0:1, 0:MAX_N],
                                 start=True,
                                 stop=(j not in banks_touched),
                                 skip_group_check=True)
            xb3 = xb[:]
            for i, (vv, t, p0, ncols, woff) in enumerate(mms):
                nc.tensor.matmul(accP[0:CS, p0:p0 + ncols],
                                 lhsT=xb3[:, vv * NTV + t, :],
                                 rhs=wS[:, woff:woff + ncols],
                                 start=False, stop=(i in stop_idx),
                                 skip_group_check=True)

            # ---------------- num copy + squares
            numS = big.tile([CS, QP], f32, tag="numS")
            nc.vector.tensor_copy(out=numS[:], in_=accP[0:CS, :])
            sqb = big.tile([CS, QP], bf16, tag="sqb")
            nc.scalar.activation(out=sqb[:], in_=accP[0:CS, :], func=AF.Square)

            # partial Sum_c num^2 via ones-matmuls (replicated over 32 rows)
            p2a = psum.tile([128, MAX_N], f32, tag="p2a")
            p2b = psum.tile([128, MAX_N], f32, tag="p2b")
            chunk_slot = []
            for j in range(QP // MAX_N):
                if j < 4:
                    dst, base = p2a, KS * j
                else:
                    dst, base = p2b, KS * (j - 4)
                nc.tensor.matmul(dst[base:base + KS, :],
                                 lhsT=oneswS[:], rhs=sqb[:, j * MAX_N:(j + 1) * MAX_N],
                                 start=True, stop=True,
                                 tile_position=(0, base),
                                 skip_group_check=True)
                chunk_slot.append((dst, base))

            # ---------------- all-reduce #2 (Sum_c num^2)
            s2a = small.tile([128, MAX_N], f32, tag="s2a")
            s2b = small.tile([128, MAX_N], f32, tag="s2b")
            nc.scalar.copy(out=s2a[:], in_=p2a[:])
            nc.scalar.copy(out=s2b[0:KS, :], in_=p2b[0:KS, :])
            sb_slot = [(s2a, 0), (s2a, 32), (s2a, 64), (s2a, 96), (s2b, 0)]
            p2_in = dram.tile([NQT, 128], f32, tag="p2_in")
            p2_out = dram.tile([NQT, 128], f32, tag="p2_out")
            for j, (src, base) in enumerate(sb_slot):
                nc.sync.dma_start(
                    out=p2_in[4 * j:4 * j + 4, :],
                    in_=src[base:base + 1, :])
            nc.gpsimd.collective_compute(
                "AllReduce", ALU.add,
                replica_groups=[list(range(NCORE))],
                ins=[p2_in[:].opt()], outs=[p2_out[:].opt()],
            )
            ssq = small.tile([128, NQT], f32, tag="ssq")
            nc.sync.dma_start(out=ssq[:], in_=p2_out[:].rearrange("t p -> p t"))

            # var2 = (r^2) * SS / C ; A = r / sqrt(var2 + eps)
            rr = small.tile([128, NQT], f32, tag="rr")
            nc.vector.tensor_tensor(out=rr[:], in0=rq[:], in1=rq[:], op=ALU.mult)
            nc.vector.tensor_tensor(out=ssq[:], in0=ssq[:], in1=rr[:], op=ALU.mult)
            nc.vector.tensor_scalar_mul(ssq[:], ssq[:], 1.0 / C)
            nc.scalar.activation(out=ssq[:], in_=ssq[:], func=AF.Sqrt,
                                 bias=epsT[:], scale=1.0)
            nc.vector.reciprocal(out=ssq[:], in_=ssq[:])
            aQ = small.tile([128, NQT], f32, tag="aQ")
            nc.vector.tensor_tensor(out=aQ[:], in0=rq[:], in1=ssq[:], op=ALU.mult)

            # ---------------- M1 / G2 (gamma, softmax(logits), beta)
            gam = rowS[0:1, 0:CS]
            bet = rowS[0:1, CS:2 * CS]
            lgt = rowS[0:1, 2 * CS:3 * CS]
            eL = small.tile([1, CS], f32, tag="eL")
            nc.scalar.activation(out=eL[:], in_=lgt, func=AF.Exp)
            sL = small.tile([1, KS], f32, tag="sL")
            nc.vector.tensor_reduce(out=sL[:],
                                    in_=eL[:].rearrange("o (k g) -> o k g", g=3),
                                    axis=mybir.AxisListType.X, op=ALU.add)
            nc.vector.reciprocal(out=sL[:], in_=sL[:])
            wgf = small.tile([1, CS], f32, tag="wgf")
            nc.vector.tensor_tensor(
                out=wgf[:].rearrange("o (k g) -> o k g", g=3),
                in0=eL[:].rearrange("o (k g) -> o k g", g=3),
                in1=sL[:].unsqueeze(2).broadcast_to([1, KS, 3]),
                op=ALU.mult)
            valsr = small.tile([1, CS], f32, tag="valsr")
            nc.vector.tensor_tensor(out=valsr[:], in0=wgf[:], in1=gam, op=ALU.mult)
            g2t = small.tile([1, CS], f32, tag="g2t")
            nc.vector.tensor_tensor(out=g2t[:], in0=wgf[:], in1=bet, op=ALU.mult)
            g2r = small.tile([1, KS], f32, tag="g2r")
            nc.vector.tensor_reduce(out=g2r[:],
                                    in_=g2t[:].rearrange("o (k g) -> o k g", g=3),
                                    axis=mybir.AxisListType.X, op=ALU.add)
            # stage vals/g2 through DRAM to change partition layout
            smallrt = dram.tile([1, 160], f32, tag="smallrt")
            nc.sync.dma_start(out=smallrt[0:1, 0:CS], in_=valsr[:])
            nc.sync.dma_start(out=smallrt[0:1, CS:CS + KS], in_=g2r[:])
            vals96 = small.tile([CS, 1], f32, tag="vals96")
            nc.sync.dma_start(out=vals96[:],
                              in_=smallrt[0:1, 0:CS].rearrange("o p -> p o"))
            g2b = small.tile([128, KS], f32, tag="g2b")
            g2_bcast = bass.AP(tensor=smallrt.tensor,
                               offset=smallrt.offset + CS,
                               ap=[[0, 128], [1, KS]])
            nc.gpsimd.dma_start(out=g2b[:], in_=g2_bcast)
            m1F = small.tile([CS, KS], f32, tag="m1F")
            nc.vector.tensor_scalar_mul(m1F[:], m1S[:], vals96[:])

            # ---------------- reducer matmuls + final scale/shift
            yps = psum.tile([128, NQT * KS], f32, tag="bigp")
            for qt in range(NQT):
                nc.tensor.matmul(yps[:, qt * KS:(qt + 1) * KS],
                                 lhsT=numS[:, qt * 128:(qt + 1) * 128],
                                 rhs=m1F[:],
                                 start=True, stop=True, skip_group_check=True)
            ySB = small.tile([128, NQT, KS], f32, tag="ySB")
            nc.vector.tensor_tensor(
                out=ySB[:], in0=yps[:].rearrange("p (t k) -> p t k", k=KS),
                in1=aQ[:].unsqueeze(2).broadcast_to([128, NQT, KS]),
                op=ALU.mult)
            nc.vector.tensor_tensor(
                out=ySB[:], in0=ySB[:],
                in1=g2b[:].unsqueeze(1).broadcast_to([128, NQT, KS]),
                op=ALU.add)
            nc.sync.dma_start(out=out_d.ap(),
                              in_=ySB[:].rearrange("p t k -> p (t k)"))

    nc.compile()
    return nc


# ------------------------------------------------------------------- driver
def make_in_maps(inputs, plan):
    lt = np.asarray(inputs["last_tokens"], np.float32)
    gamma = np.asarray(inputs["post_gamma"], np.float32).ravel()
    beta = np.asarray(inputs["post_beta"], np.float32).ravel()
    logits = np.asarray(inputs["logits"], np.float32)
    w_view = np.asarray(inputs["w_view"], np.float32).ravel()
    Hp, Wp = int(inputs["patch_h"]), int(inputs["patch_w"])

    NT_V = plan["NT_V"]
    NTT = V * NT_V
    tokt = retile_tokens(lt, NT_V, Hp, Wp)          # (128, NTT, 768)
    wmat_b = plan["wmat"].astype(ml_dtypes.bfloat16)
    cnt_flat = np.ascontiguousarray(
        plan["cnt_perm"].reshape(128, NQT * V), np.float32)

    m1mask = np.zeros((CS, KS), np.float32)
    m1mask[np.arange(CS), np.arange(CS) // 3] = 1.0
    onesw = np.ones((CS, KS), ml_dtypes.bfloat16)
    zrow = np.zeros((1, 512), ml_dtypes.bfloat16)

    in_maps = []
    for k in range(NCORE):
        rowc = np.zeros((1, 640), np.float32)
        rowc[0, 0:CS] = gamma[CS * k:CS * (k + 1)]
        rowc[0, CS:2 * CS] = beta[CS * k:CS * (k + 1)]
        rowc[0, 2 * CS:3 * CS] = logits[KS * k:KS * (k + 1)].reshape(-1)
        rowc[0, 288:288 + V] = w_view
        tok_k = np.ascontiguousarray(
            tokt[:, :, CS * k:CS * (k + 1)].reshape(128, NTT * CS), np.float32)
        in_maps.append({
            "tok": tok_k,
            "wmat": np.ascontiguousarray(wmat_b),
            "cnt": cnt_flat,
            "rowc": rowc,
            "m1mask": m1mask,
            "onesw": onesw,
            "zrow": zrow,
        })
    return in_maps


def assemble_output(results, plan):
    Y = np.zeros((Q, C_CTX), np.float32)
    tmp = np.zeros((QP, C_CTX), np.float32)
    for k in range(NCORE):
        arr = np.asarray(results[k]["out"], np.float32).reshape(128, NQT, KS)
        tmp[:, KS * k:KS * (k + 1)] = arr.transpose(1, 0, 2).reshape(QP, KS)
    Y[plan["perm"]] = tmp[:Q]
    return np.ascontiguousarray(
        Y.reshape(1, BEV_H, BEV_W, C_CTX).transpose(0, 3, 1, 2))


_CACHE = {}


def _get_program(lidar2img, patch_h, patch_w):
    key = (lidar2img.tobytes(), int(patch_h), int(patch_w))
    if key not in _CACHE:
        plan = build_plan(lidar2img, patch_h, patch_w)
        NTT = V * plan["NT_V"]
        WCOLS = plan["wmat"].shape[1]
        nc = build_program(NTT, WCOLS, plan["mms"])
        _CACHE[key] = (plan, nc)
    return _CACHE[key]


def _install_ntff_shim():
    """Provide antenv.axon_hooks (absent in this image) so trace=True can
    capture NTFF profiles via the axon PJRT .so. Used only by test.py."""
    import types
    import ctypes
    import contextlib
    if "antenv.axon_hooks" in sys.modules:
        return
    so_path = "/opt/axon/libaxon_pjrt.so"
    lib = ctypes.CDLL(so_path)
    if not hasattr(lib, "axon_start_nrt_profile"):
        return
    lib.axon_start_nrt_profile.argtypes = [
        ctypes.POINTER(ctypes.c_int64), ctypes.c_size_t]
    lib.axon_start_nrt_profile.restype = ctypes.c_int64
    lib.axon_stop_nrt_profile.argtypes = [ctypes.c_char_p]
    lib.axon_stop_nrt_profile.restype = ctypes.c_int64

    @contextlib.contextmanager
    def _hook(output_dir, device_ids):
        import jax
        jax.devices()
        if device_ids:
            ids = (ctypes.c_int64 * len(device_ids))(*device_ids)
            rc = lib.axon_start_nrt_profile(ids, len(device_ids))
        else:
            rc = lib.axon_start_nrt_profile(None, 0)
        if rc != 0:
            raise RuntimeError(f"axon_start_nrt_profile rc={rc}")
        try:
            yield
        finally:
            n = lib.axon_stop_nrt_profile(str(output_dir).encode())
            print(f"ntff profile: {n} file(s) -> {output_dir}", file=sys.stderr)

    mod = types.ModuleType("antenv.axon_hooks")
    mod.get_axon_ntff_profile_hook = lambda: _hook
    mod.set_axon_ntff_profile_hook = lambda h: None
    sys.modules["antenv.axon_hooks"] = mod
    import antenv
    antenv.axon_hooks = mod


def kernel(last_tokens, lidar2img, w_view, post_gamma, post_beta, logits,
           patch_h, patch_w, _trace=False):
    import concourse.bass_utils as bu
    from concourse.bass_utils import run_bass_kernel_spmd
    if _trace:
        _install_ntff_shim()
        bu.upload_artifacts = lambda tmpdir: "local://" + str(tmpdir)
    inputs = dict(last_tokens=np.asarray(last_tokens),
                  lidar2img=np.asarray(lidar2img, np.float32),
                  w_view=w_view, post_gamma=post_gamma, post_beta=post_beta,
                  logits=logits, patch_h=patch_h, patch_w=patch_w)
    plan, nc = _get_program(inputs["lidar2img"], patch_h, patch_w)
    in_maps = make_in_maps(inputs, plan)
    res = run_bass_kernel_spmd(nc, in_maps, core_ids=list(range(NCORE)),
                               trace=_trace)
    out = assemble_output(res.results, plan)
    kernel.last_result = res
    return out
